# revision 28
# baseline (speedup 1.0000x reference)
"""CrossAttention Trainium2 kernel, v4.

Sharding: 8 cores = 4 batches x 2 head-groups. Each core computes one
batch's attention for 8 of the 16 heads.

Per-core layout (B=4, T=2048, HIN=1024, H=16, E=64):
  - inputs arrive natural bf16: xq/xkv [2048, 1024]; weights pre-scaled
    bf16 [128, 8, 512] (partition-major hin blocks).
  - x -> x^T on device via 16 xbar transpose-DMAs into SBUF.
  - projections: qT/kT [128, 4(pair), 2048] bf16 (heads packed 2 per
    128 partitions); v natural [128(t), 16(blk), 8(head), 96] with ones
    columns 0:32 (softmax denominator) and v at columns 32:96.
  - attention per (pair, q-chunk of 1024): S^T = K^T.T @ Q^T into PSUM
    fp32, exp on ScalarE -> P bf16, O^T accumulated [96, 1024] fp32
    (row 0 = denominator, rows 32:96 = O). Normalize via reciprocal +
    GpSimd partition-broadcast + one multiply; outputs written bf16.
"""

import numpy as np
import ml_dtypes

import concourse.bass as bass
import concourse.mybir as mybir
import concourse.tile as tile
from concourse import bacc
from concourse.bass_utils import run_bass_kernel_spmd
from concourse.tile import TileContext, ScopedClock

BF16 = mybir.dt.bfloat16
F32 = mybir.dt.float32

B, T, HIN, H, E = 4, 2048, 1024, 16, 64
NCORES = 8
HG = 2                    # head groups
HPC = H // HG             # heads per core = 8
ES = HPC * E              # 512 (he-slice width per core)
NP = HPC // 2             # head pairs per core = 4
KI = HIN // 128           # 8 hin-blocks
NBLK = T // 128           # 16 t-blocks
QC = 1024                 # q-chunk width
NQC = T // QC             # 2
SCALE = float(E) ** -0.25
# O^T row layout: row 0 = softmax denominator (ones column), rows 64:128 = O.
# The 63-row gap keeps reciprocal/partition_broadcast sourced at partition 0
# (HW breaks them when sourced at partition 64) while the consumer ops sit
# on a legal 64-partition range at base 64 (base-32 ranges are capped at 32
# partitions by the BIR verifier).
VOFF = 64
VW = VOFF + E             # 128 = lhsT width of the O matmul

_EXP = mybir.ActivationFunctionType.Exp

# tuning knobs (settable by bench scripts before build_nc)
PP_BUFS = 8
NORM_MODE = "full"        # "full" | "simple" (timing-only, wrong output)


def _patch_tail_drain():
    """walrus in this container allows only ONE sync-wait per instruction;
    Tile's kernel-tail drain accumulates one wait per live proc. Spread the
    waits across single-wait NOPs."""
    if getattr(TileContext, "_tail_drain_patched", False):
        return

    def _drain_and_barrier(self, tick_clock, wait_clock):
        probe = self.nc.sync.nop(nofuse=True, hint="tail_wait_probe")
        wait_clock.add_sem_waits(
            probe.ins, ScopedClock({None: tick_clock.global_clock})
        )
        si = probe.ins.sync_info
        waits = list(si.on_wait) if si is not None else []
        if len(waits) > 1:
            probe.ins.sync_info = mybir.SyncInfo(
                on_wait=waits[:1], on_update=list(si.on_update)
            )
            for i in range(1, len(waits)):
                n2 = self.nc.sync.nop(nofuse=True, hint=f"tail_wait_{i}")
                n2.ins.sync_info = mybir.SyncInfo(on_wait=[waits[i]], on_update=[])
        self.nc.sync.drain()
        self.nc.all_engine_barrier()
        popped = self.nc._tile_sem_poison_stack.pop()
        assert popped is self._sem_poison
        self.nc.clear_and_free_semaphores(list(self.sems.allocated().values()))
        self.nc.all_engine_barrier()

    TileContext._drain_and_barrier = _drain_and_barrier
    TileContext._tail_drain_patched = True


def emit_proj(nc, tc, qT, kT, v_sb, xq, xkv, wq, wk, wv):
        nc.vector.memset(v_sb, 1.0)

        # ---------------- Phase 1: projections ----------------
        with tc.tile_pool(name="xin", bufs=1) as xin, \
             tc.tile_pool(name="win", bufs=1) as win, \
             tc.tile_pool(name="ps_qk", bufs=2, space="PSUM") as ps_qk, \
             tc.tile_pool(name="ps_v", bufs=2, space="PSUM") as ps_v:
            wq_sb = win.tile([128, KI, ES], BF16, tag="wq")
            wk_sb = win.tile([128, KI, ES], BF16, tag="wk")
            wv_sb = win.tile([128, KI, ES], BF16, tag="wv")
            nc.sync.dma_start(out=wq_sb, in_=wq[:, :, :])
            nc.sync.dma_start(out=wk_sb, in_=wk[:, :, :])
            nc.sync.dma_start(out=wv_sb, in_=wv[:, :, :])

            xq_sb = xin.tile([128, KI, T], BF16, tag="xq")
            xkv_sb = xin.tile([128, KI, T], BF16, tag="xkv")
            for i in range(KI):
                nc.sync.dma_start_transpose(
                    xq_sb[:, i, :], xq[:, i * 128:(i + 1) * 128])
                nc.sync.dma_start_transpose(
                    xkv_sb[:, i, :], xkv[:, i * 128:(i + 1) * 128])

            # q^T / k^T: [he(128-pair), t] = W_slice.T @ x^T
            for w_sb, x_sb, dst in ((wq_sb, xq_sb, qT), (wk_sb, xkv_sb, kT)):
                for p in range(NP):
                    mp = slice(p * 128, (p + 1) * 128)
                    for tc2 in range(T // 1024):
                        ps = ps_qk.tile([128, 1024], F32, tag="psqk")
                        for i in range(KI):
                            for ns in range(2):
                                c0 = tc2 * 1024 + ns * 512
                                nc.tensor.matmul(
                                    ps[:, ns * 512:(ns + 1) * 512],
                                    w_sb[:, i, mp],
                                    x_sb[:, i, c0:c0 + 512],
                                    start=(i == 0), stop=(i == KI - 1))
                        nc.vector.tensor_copy(
                            dst[:, p, tc2 * 1024:(tc2 + 1) * 1024], ps)

            # v natural: [t(128-blk), he] = x_blk.T @ W  (x^T as stationary)
            for tb in range(NBLK):
                psv = ps_v.tile([128, ES], F32, tag="psv")
                for i in range(KI):
                    nc.tensor.matmul(
                        psv, xkv_sb[:, i, tb * 128:(tb + 1) * 128],
                        wv_sb[:, i, :], start=(i == 0), stop=(i == KI - 1))
                nc.vector.tensor_copy(
                    v_sb[:, tb, :, VOFF:VW],
                    psv.rearrange("p (h e) -> p h e", h=HPC))

def emit_attn(nc, tc, qT, kT, v_sb, out):
        # ---------------- Phase 2: attention ----------------
        with tc.tile_pool(name="pp_x", bufs=3, space="PSUM") as pp_x, \
             tc.tile_pool(name="pp_o", bufs=1, space="PSUM") as pp_o, \
             tc.tile_pool(name="pP", bufs=PP_BUFS) as pP, \
             tc.tile_pool(name="pn", bufs=2) as pn, \
             tc.tile_pool(name="po", bufs=2) as po:
            for p in range(NP):
                for qc in range(NQC):
                    for h in range(2):
                        q0 = qc * QC
                        hp = slice(h * E, (h + 1) * E)
                        oT = pp_o.tile([VW, QC], F32, tag="ot")
                        for kt in range(NBLK):
                            k0 = kt * 128
                            # triple-buffered X: S(kt+1) never waits on
                            # exp(kt), the ACT stream stays dense
                            X = pp_x.tile([128, QC], F32, tag="X")
                            for ns in range(QC // 512):
                                nc.tensor.matmul(
                                    X[:, ns * 512:(ns + 1) * 512],
                                    kT[hp, p, k0:k0 + 128],
                                    qT[hp, p, q0 + ns * 512:q0 + (ns + 1) * 512],
                                    start=True, stop=True)
                            Pt = pP.tile([128, QC], BF16, tag="P")
                            nc.scalar.activation(out=Pt, in_=X, func=_EXP)
                            vp = v_sb[:, kt, 2 * p + h, :]
                            for ns in range(QC // 512):
                                nc.tensor.matmul(
                                    oT[:, ns * 512:(ns + 1) * 512],
                                    vp,
                                    Pt[:, ns * 512:(ns + 1) * 512],
                                    start=(kt == 0), stop=(kt == NBLK - 1))
                        # evacuate oT to SBUF immediately (single PSUM slot:
                        # the next pass's O-matmuls wait on this release)
                        ocp = pn.tile([VW, QC], F32, tag="ocp")
                        nc.vector.tensor_copy(ocp, oT)
                        # normalize: out = ocp[64:128] * (1/ocp[0])
                        osb = po.tile([VW, QC], BF16, tag="osb")
                        if NORM_MODE == "full":
                            rr = pn.tile([1, QC], F32, tag="rr")
                            nc.vector.reciprocal_approx_fast(
                                out=rr, in_=ocp[0:1, :])
                            rb = pn.tile([VW, QC], F32, tag="rb")
                            # HW: partition_broadcast must WRITE at base 0
                            # (base-64 dst ranges come back wrong), so fill
                            # all 128 partitions and read 64:128 below.
                            nc.gpsimd.partition_broadcast(
                                rb, rr[0:1, :], channels=VW)
                            nc.vector.tensor_mul(
                                osb[VOFF:VW, :], ocp[VOFF:VW, :],
                                rb[VOFF:VW, :])
                        else:  # timing-only variant without the recip chain
                            nc.vector.tensor_copy(
                                osb[VOFF:VW, :], ocp[VOFF:VW, :])
                        nc.sync.dma_start(
                            out=out[p, h, :, q0:q0 + QC], in_=osb[VOFF:VW, :])


def build_nc(reps: int = 1, loop: int = 0, debug: bool = False,
             phase: str = "all"):
    """reps: python-unrolled body repetitions. loop: if >0, wrap one body
    in a hardware For_i loop with `loop` iterations (for timing)."""
    _patch_tail_drain()
    nc = bacc.Bacc(None)
    xq = nc.declare_dram_parameter("xq", [T, HIN], BF16, isOutput=False)
    xkv = nc.declare_dram_parameter("xkv", [T, HIN], BF16, isOutput=False)
    wq = nc.declare_dram_parameter("wq", [128, KI, ES], BF16, isOutput=False)
    wk = nc.declare_dram_parameter("wk", [128, KI, ES], BF16, isOutput=False)
    wv = nc.declare_dram_parameter("wv", [128, KI, ES], BF16, isOutput=False)
    out = nc.declare_dram_parameter("out", [NP, 2, E, T], BF16, isOutput=True)
    dbg = None
    if debug:
        dbg = {
            "dq": nc.declare_dram_parameter("dq", [128, NP, T], BF16, isOutput=True),
            "dk": nc.declare_dram_parameter("dk", [128, NP, T], BF16, isOutput=True),
            "dv": nc.declare_dram_parameter(
                "dv", [128, NBLK, HPC, VW], BF16, isOutput=True),
            "dP": nc.declare_dram_parameter("dP", [128, 2, QC], BF16, isOutput=True),
        }
    with TileContext(nc) as tc:
        with tc.tile_pool(name="persist", bufs=1) as persist:
            qT = persist.tile([128, NP, T], BF16)   # [pair-rows, pair, t]
            kT = persist.tile([128, NP, T], BF16)
            v_sb = persist.tile([128, NBLK, HPC, VW], BF16)
            if loop:
                if phase == "all":
                    with tc.For_i(0, loop, 1):
                        emit_proj(nc, tc, qT, kT, v_sb, xq, xkv, wq, wk, wv)
                        emit_attn(nc, tc, qT, kT, v_sb, out)
                elif phase == "proj":
                    with tc.For_i(0, loop, 1):
                        emit_proj(nc, tc, qT, kT, v_sb, xq, xkv, wq, wk, wv)
                elif phase == "attn":
                    emit_proj(nc, tc, qT, kT, v_sb, xq, xkv, wq, wk, wv)
                    with tc.For_i(0, loop, 1):
                        emit_attn(nc, tc, qT, kT, v_sb, out)
                else:
                    raise ValueError(phase)
            else:
                for _ in range(reps):
                    emit_proj(nc, tc, qT, kT, v_sb, xq, xkv, wq, wk, wv)
                    if dbg is not None:
                        nc.sync.dma_start(out=dbg["dq"][:, :, :], in_=qT)
                        nc.sync.dma_start(out=dbg["dk"][:, :, :], in_=kT)
                        nc.sync.dma_start(out=dbg["dv"][:, :, :, :], in_=v_sb)
                    emit_attn(nc, tc, qT, kT, v_sb, out)
    nc.finalize()
    return nc


def make_in_maps(query, key_value, Wq, Wk, Wv):
    """Host-side sharding: cast to bf16, slice weights per head-group."""
    bf = ml_dtypes.bfloat16
    qbf = query.astype(bf)
    kvbf = key_value.astype(bf)
    wq_s = (Wq.astype(np.float32) * SCALE).astype(bf)
    wk_s = (Wk.astype(np.float32) * SCALE).astype(bf)
    wv_s = Wv.astype(bf)
    def wprep(w, cols):
        # [HIN, ES] -> [128(p), KI(i), ES], row i*128+p of W at [p, i, :]
        return np.ascontiguousarray(
            w[:, cols].reshape(KI, 128, ES).transpose(1, 0, 2))

    wslices = []
    for hg in range(HG):
        cols = slice(hg * ES, (hg + 1) * ES)
        wslices.append({
            "wq": wprep(wq_s, cols),
            "wk": wprep(wk_s, cols),
            "wv": wprep(wv_s, cols),
        })
    in_maps = []
    for c in range(NCORES):
        b, hg = divmod(c, HG)
        in_maps.append({
            "xq": qbf[b],
            "xkv": kvbf[b],
            **wslices[hg],
        })
    return in_maps


def assemble_output(results):
    full = np.empty((B, T, H * E), dtype=np.float32)
    for c in range(NCORES):
        b, hg = divmod(c, HG)
        o = np.asarray(results[c]["out"])          # [NP, 2, E, T] bf16
        o32 = np.ascontiguousarray(o.reshape(ES, T).astype(np.float32).T)
        full[b, :, hg * ES:(hg + 1) * ES] = o32
    return full


_NC_CACHE = {}


def kernel(query, key_value, Wq, Wk, Wv):
    query = np.asarray(query, dtype=np.float32)
    key_value = np.asarray(key_value, dtype=np.float32)
    Wq = np.asarray(Wq, dtype=np.float32)
    Wk = np.asarray(Wk, dtype=np.float32)
    Wv = np.asarray(Wv, dtype=np.float32)

    if "nc" not in _NC_CACHE:
        _NC_CACHE["nc"] = build_nc(reps=1)
    nc = _NC_CACHE["nc"]
    in_maps = make_in_maps(query, key_value, Wq, Wk, Wv)
    res = run_bass_kernel_spmd(nc, in_maps, list(range(NCORES)))
    return assemble_output(res.results)


if __name__ == "__main__":
    rng = np.random.default_rng(0)
    q = rng.standard_normal((B, T, HIN), dtype=np.float32)
    kv = rng.standard_normal((B, T, HIN), dtype=np.float32)
    s = 1.0 / np.sqrt(HIN)
    wq = rng.uniform(-s, s, (HIN, H * E)).astype(np.float32)
    wk = rng.uniform(-s, s, (HIN, H * E)).astype(np.float32)
    wv = rng.uniform(-s, s, (HIN, H * E)).astype(np.float32)
    out = kernel(query=q, key_value=kv, Wq=wq, Wk=wk, Wv=wv)
    print("out", out.shape, out.dtype, np.abs(out).mean())


# revision 29
# speedup vs baseline: 1.1021x; 1.1021x over previous
"""CrossAttention Trainium2 kernel, v4.

Sharding: 8 cores = 4 batches x 2 head-groups. Each core computes one
batch's attention for 8 of the 16 heads.

Per-core layout (B=4, T=2048, HIN=1024, H=16, E=64):
  - inputs arrive natural bf16: xq/xkv [2048, 1024]; weights pre-scaled
    bf16 [128, 8, 512] (partition-major hin blocks).
  - x -> x^T on device via 16 xbar transpose-DMAs into SBUF.
  - projections: qT/kT [128, 4(pair), 2048] bf16 (heads packed 2 per
    128 partitions); v natural [128(t), 16(blk), 8(head), 96] with ones
    columns 0:32 (softmax denominator) and v at columns 32:96.
  - attention per (pair, q-chunk of 1024): S^T = K^T.T @ Q^T into PSUM
    fp32, exp on ScalarE -> P bf16, O^T accumulated [96, 1024] fp32
    (row 0 = denominator, rows 32:96 = O). Normalize via reciprocal +
    GpSimd partition-broadcast + one multiply; outputs written bf16.
"""

import numpy as np
import ml_dtypes

import concourse.bass as bass
import concourse.mybir as mybir
import concourse.tile as tile
from concourse import bacc
from concourse.bass_utils import run_bass_kernel_spmd
from concourse.tile import TileContext, ScopedClock

BF16 = mybir.dt.bfloat16
F32 = mybir.dt.float32

B, T, HIN, H, E = 4, 2048, 1024, 16, 64
NCORES = 8
HG = 2                    # head groups
HPC = H // HG             # heads per core = 8
ES = HPC * E              # 512 (he-slice width per core)
NP = HPC // 2             # head pairs per core = 4
KI = HIN // 128           # 8 hin-blocks
NBLK = T // 128           # 16 t-blocks
QC = 1024                 # q-chunk width
NQC = T // QC             # 2
SCALE = float(E) ** -0.25
# O^T row layout: row 0 = softmax denominator (ones column), rows 64:128 = O.
# The 63-row gap keeps reciprocal/partition_broadcast sourced at partition 0
# (HW breaks them when sourced at partition 64) while the consumer ops sit
# on a legal 64-partition range at base 64 (base-32 ranges are capped at 32
# partitions by the BIR verifier).
VOFF = 64
VW = VOFF + E             # 128 = lhsT width of the O matmul

_EXP = mybir.ActivationFunctionType.Exp

# tuning knobs (settable by bench scripts before build_nc)
PP_BUFS = 8
NORM_MODE = "full"        # "full" | "simple" (timing-only, wrong output)


def _patch_tail_drain():
    """walrus in this container allows only ONE sync-wait per instruction;
    Tile's kernel-tail drain accumulates one wait per live proc. Spread the
    waits across single-wait NOPs."""
    if getattr(TileContext, "_tail_drain_patched", False):
        return

    def _drain_and_barrier(self, tick_clock, wait_clock):
        probe = self.nc.sync.nop(nofuse=True, hint="tail_wait_probe")
        wait_clock.add_sem_waits(
            probe.ins, ScopedClock({None: tick_clock.global_clock})
        )
        si = probe.ins.sync_info
        waits = list(si.on_wait) if si is not None else []
        if len(waits) > 1:
            probe.ins.sync_info = mybir.SyncInfo(
                on_wait=waits[:1], on_update=list(si.on_update)
            )
            for i in range(1, len(waits)):
                n2 = self.nc.sync.nop(nofuse=True, hint=f"tail_wait_{i}")
                n2.ins.sync_info = mybir.SyncInfo(on_wait=[waits[i]], on_update=[])
        self.nc.sync.drain()
        self.nc.all_engine_barrier()
        popped = self.nc._tile_sem_poison_stack.pop()
        assert popped is self._sem_poison
        self.nc.clear_and_free_semaphores(list(self.sems.allocated().values()))
        self.nc.all_engine_barrier()

    TileContext._drain_and_barrier = _drain_and_barrier
    TileContext._tail_drain_patched = True


def emit_proj(nc, tc, qT, kT, v_sb, xq, xkv, wq, wk, wv):
        nc.vector.memset(v_sb, 1.0)

        # ---------------- Phase 1: projections ----------------
        with tc.tile_pool(name="xin", bufs=1) as xin, \
             tc.tile_pool(name="win", bufs=1) as win, \
             tc.tile_pool(name="ps_qk", bufs=2, space="PSUM") as ps_qk, \
             tc.tile_pool(name="ps_v", bufs=2, space="PSUM") as ps_v:
            wq_sb = win.tile([128, KI, ES], BF16, tag="wq")
            wk_sb = win.tile([128, KI, ES], BF16, tag="wk")
            wv_sb = win.tile([128, KI, ES], BF16, tag="wv")
            nc.sync.dma_start(out=wq_sb, in_=wq[:, :, :])
            nc.sync.dma_start(out=wk_sb, in_=wk[:, :, :])
            nc.sync.dma_start(out=wv_sb, in_=wv[:, :, :])

            xq_sb = xin.tile([128, KI, T], BF16, tag="xq")
            xkv_sb = xin.tile([128, KI, T], BF16, tag="xkv")
            for i in range(KI):
                nc.sync.dma_start_transpose(
                    xq_sb[:, i, :], xq[:, i * 128:(i + 1) * 128])
                nc.sync.dma_start_transpose(
                    xkv_sb[:, i, :], xkv[:, i * 128:(i + 1) * 128])

            # q^T / k^T: [he(128-pair), t] = W_slice.T @ x^T
            for w_sb, x_sb, dst in ((wq_sb, xq_sb, qT), (wk_sb, xkv_sb, kT)):
                for p in range(NP):
                    mp = slice(p * 128, (p + 1) * 128)
                    for tc2 in range(T // 1024):
                        ps = ps_qk.tile([128, 1024], F32, tag="psqk")
                        for i in range(KI):
                            for ns in range(2):
                                c0 = tc2 * 1024 + ns * 512
                                nc.tensor.matmul(
                                    ps[:, ns * 512:(ns + 1) * 512],
                                    w_sb[:, i, mp],
                                    x_sb[:, i, c0:c0 + 512],
                                    start=(i == 0), stop=(i == KI - 1))
                        nc.vector.tensor_copy(
                            dst[:, p, tc2 * 1024:(tc2 + 1) * 1024], ps)

            # v natural: [t(128-blk), he] = x_blk.T @ W  (x^T as stationary)
            for tb in range(NBLK):
                psv = ps_v.tile([128, ES], F32, tag="psv")
                for i in range(KI):
                    nc.tensor.matmul(
                        psv, xkv_sb[:, i, tb * 128:(tb + 1) * 128],
                        wv_sb[:, i, :], start=(i == 0), stop=(i == KI - 1))
                nc.vector.tensor_copy(
                    v_sb[:, tb, :, VOFF:VW],
                    psv.rearrange("p (h e) -> p h e", h=HPC))

def emit_attn(nc, tc, qT, kT, v_sb, out):
        # ---------------- Phase 2: attention ----------------
        with tc.tile_pool(name="pp_x", bufs=2, space="PSUM") as pp_x, \
             tc.tile_pool(name="pp_o", bufs=2, space="PSUM") as pp_o, \
             tc.tile_pool(name="pP", bufs=PP_BUFS) as pP, \
             tc.tile_pool(name="pn", bufs=2) as pn, \
             tc.tile_pool(name="po", bufs=2) as po:
            for p in range(NP):
                for qc in range(NQC):
                    q0 = qc * QC
                    oT = [pp_o.tile([VW, QC], F32, tag="ot",
                                    name=f"oT{p}_{qc}_{h}") for h in range(2)]
                    for kt in range(NBLK):
                        k0 = kt * 128
                        for h in range(2):
                            hp = slice(h * E, (h + 1) * E)
                            # per-head X, double-buffered: S of one head
                            # overlaps exp of the other
                            X = pp_x.tile([128, QC], F32, tag="X")
                            for ns in range(QC // 512):
                                nc.tensor.matmul(
                                    X[:, ns * 512:(ns + 1) * 512],
                                    kT[hp, p, k0:k0 + 128],
                                    qT[hp, p, q0 + ns * 512:q0 + (ns + 1) * 512],
                                    start=True, stop=True)
                            Pt = pP.tile([128, QC], BF16, tag="P")
                            nc.scalar.activation(out=Pt, in_=X, func=_EXP)
                            vp = v_sb[:, kt, 2 * p + h, :]
                            for ns in range(QC // 512):
                                nc.tensor.matmul(
                                    oT[h][:, ns * 512:(ns + 1) * 512],
                                    vp,
                                    Pt[:, ns * 512:(ns + 1) * 512],
                                    start=(kt == 0), stop=(kt == NBLK - 1))
                    # normalize: out = oT[64:128] * (1/oT[0])
                    for h in range(2):
                        osb = po.tile([VW, QC], BF16, tag="osb")
                        if NORM_MODE == "full":
                            rr = pn.tile([1, QC], F32, tag="rr", name=f"rr{h}")
                            nc.vector.reciprocal_approx_fast(
                                out=rr, in_=oT[h][0:1, :])
                            rb = pn.tile([VW, QC], F32, tag="rb", name=f"rb{h}")
                            # HW: partition_broadcast must WRITE at base 0
                            # (base-64 dst ranges come back wrong), so fill
                            # all 128 partitions and read 64:128 below.
                            nc.gpsimd.partition_broadcast(
                                rb, rr[0:1, :], channels=VW)
                            nc.vector.tensor_mul(
                                osb[VOFF:VW, :], oT[h][VOFF:VW, :],
                                rb[VOFF:VW, :])
                        else:  # timing-only variant without the recip chain
                            nc.vector.tensor_copy(
                                osb[VOFF:VW, :], oT[h][VOFF:VW, :])
                        nc.sync.dma_start(
                            out=out[p, h, :, q0:q0 + QC], in_=osb[VOFF:VW, :])


def build_nc(reps: int = 1, loop: int = 0, debug: bool = False,
             phase: str = "all"):
    """reps: python-unrolled body repetitions. loop: if >0, wrap one body
    in a hardware For_i loop with `loop` iterations (for timing)."""
    _patch_tail_drain()
    nc = bacc.Bacc(None)
    xq = nc.declare_dram_parameter("xq", [T, HIN], BF16, isOutput=False)
    xkv = nc.declare_dram_parameter("xkv", [T, HIN], BF16, isOutput=False)
    wq = nc.declare_dram_parameter("wq", [128, KI, ES], BF16, isOutput=False)
    wk = nc.declare_dram_parameter("wk", [128, KI, ES], BF16, isOutput=False)
    wv = nc.declare_dram_parameter("wv", [128, KI, ES], BF16, isOutput=False)
    out = nc.declare_dram_parameter("out", [NP, 2, E, T], BF16, isOutput=True)
    dbg = None
    if debug:
        dbg = {
            "dq": nc.declare_dram_parameter("dq", [128, NP, T], BF16, isOutput=True),
            "dk": nc.declare_dram_parameter("dk", [128, NP, T], BF16, isOutput=True),
            "dv": nc.declare_dram_parameter(
                "dv", [128, NBLK, HPC, VW], BF16, isOutput=True),
            "dP": nc.declare_dram_parameter("dP", [128, 2, QC], BF16, isOutput=True),
        }
    with TileContext(nc) as tc:
        with tc.tile_pool(name="persist", bufs=1) as persist:
            qT = persist.tile([128, NP, T], BF16)   # [pair-rows, pair, t]
            kT = persist.tile([128, NP, T], BF16)
            v_sb = persist.tile([128, NBLK, HPC, VW], BF16)
            if loop:
                if phase == "all":
                    with tc.For_i(0, loop, 1):
                        emit_proj(nc, tc, qT, kT, v_sb, xq, xkv, wq, wk, wv)
                        emit_attn(nc, tc, qT, kT, v_sb, out)
                elif phase == "proj":
                    with tc.For_i(0, loop, 1):
                        emit_proj(nc, tc, qT, kT, v_sb, xq, xkv, wq, wk, wv)
                elif phase == "attn":
                    emit_proj(nc, tc, qT, kT, v_sb, xq, xkv, wq, wk, wv)
                    with tc.For_i(0, loop, 1):
                        emit_attn(nc, tc, qT, kT, v_sb, out)
                else:
                    raise ValueError(phase)
            else:
                for _ in range(reps):
                    emit_proj(nc, tc, qT, kT, v_sb, xq, xkv, wq, wk, wv)
                    if dbg is not None:
                        nc.sync.dma_start(out=dbg["dq"][:, :, :], in_=qT)
                        nc.sync.dma_start(out=dbg["dk"][:, :, :], in_=kT)
                        nc.sync.dma_start(out=dbg["dv"][:, :, :, :], in_=v_sb)
                    emit_attn(nc, tc, qT, kT, v_sb, out)
    nc.finalize()
    return nc


def make_in_maps(query, key_value, Wq, Wk, Wv):
    """Host-side sharding: cast to bf16, slice weights per head-group."""
    bf = ml_dtypes.bfloat16
    qbf = query.astype(bf)
    kvbf = key_value.astype(bf)
    wq_s = (Wq.astype(np.float32) * SCALE).astype(bf)
    wk_s = (Wk.astype(np.float32) * SCALE).astype(bf)
    wv_s = Wv.astype(bf)
    def wprep(w, cols):
        # [HIN, ES] -> [128(p), KI(i), ES], row i*128+p of W at [p, i, :]
        return np.ascontiguousarray(
            w[:, cols].reshape(KI, 128, ES).transpose(1, 0, 2))

    wslices = []
    for hg in range(HG):
        cols = slice(hg * ES, (hg + 1) * ES)
        wslices.append({
            "wq": wprep(wq_s, cols),
            "wk": wprep(wk_s, cols),
            "wv": wprep(wv_s, cols),
        })
    in_maps = []
    for c in range(NCORES):
        b, hg = divmod(c, HG)
        in_maps.append({
            "xq": qbf[b],
            "xkv": kvbf[b],
            **wslices[hg],
        })
    return in_maps


def assemble_output(results):
    full = np.empty((B, T, H * E), dtype=np.float32)
    for c in range(NCORES):
        b, hg = divmod(c, HG)
        o = np.asarray(results[c]["out"])          # [NP, 2, E, T] bf16
        o32 = np.ascontiguousarray(o.reshape(ES, T).astype(np.float32).T)
        full[b, :, hg * ES:(hg + 1) * ES] = o32
    return full


_NC_CACHE = {}


def kernel(query, key_value, Wq, Wk, Wv):
    query = np.asarray(query, dtype=np.float32)
    key_value = np.asarray(key_value, dtype=np.float32)
    Wq = np.asarray(Wq, dtype=np.float32)
    Wk = np.asarray(Wk, dtype=np.float32)
    Wv = np.asarray(Wv, dtype=np.float32)

    if "nc" not in _NC_CACHE:
        _NC_CACHE["nc"] = build_nc(reps=1)
    nc = _NC_CACHE["nc"]
    in_maps = make_in_maps(query, key_value, Wq, Wk, Wv)
    res = run_bass_kernel_spmd(nc, in_maps, list(range(NCORES)))
    return assemble_output(res.results)


if __name__ == "__main__":
    rng = np.random.default_rng(0)
    q = rng.standard_normal((B, T, HIN), dtype=np.float32)
    kv = rng.standard_normal((B, T, HIN), dtype=np.float32)
    s = 1.0 / np.sqrt(HIN)
    wq = rng.uniform(-s, s, (HIN, H * E)).astype(np.float32)
    wk = rng.uniform(-s, s, (HIN, H * E)).astype(np.float32)
    wv = rng.uniform(-s, s, (HIN, H * E)).astype(np.float32)
    out = kernel(query=q, key_value=kv, Wq=wq, Wk=wk, Wv=wv)
    print("out", out.shape, out.dtype, np.abs(out).mean())
